# revision 52
# baseline (speedup 1.0000x reference)
"""GATv2 layer (broadcast-score variant) as a Bass/Tile kernel on 8 NeuronCores.

Math: since scores[i,j] = e[j] (row-broadcast) masked by A, the masked softmax +
aggregation collapse to
    g = exp(e),  e = relu(X @ W.T) @ a_w
    out = relu( (A @ (g*Wh)) / (A @ g) )          with Wh = X @ W.T
Each core computes a 1024-row block of the output:
  phase 1 (replicated): Wh, e, g, G = [g*Wh | g]  ([8192, 129])
  phase 2 (sharded):    acc = A_block @ G  via PE, contraction j on partitions,
                        using the host-transposed A.T block as lhsT.
"""

import numpy as np

import concourse.tile as tile
from concourse import bacc, mybir
from concourse.bass_utils import run_bass_kernel_spmd

N, IN_DIM, OUT_DIM = 8192, 256, 128
NCORES = 8
RPC = N // NCORES          # rows per core (1024)
P = 128                    # partitions
NJ = N // P                # 64 contraction chunks
NI = RPC // P              # 8 output row-tiles per core
DH = IN_DIM // P           # 2 chunks of the d-contraction
import os

F32 = mybir.dt.float32
F32R = mybir.dt.float32r   # TF32-like: 1 cyc/row on PE when moving dim >= 256
BF16 = mybir.dt.bfloat16
# phase-2 matmul dtype: f32r, f32, bf16
PH2_DT = {"f32r": F32R, "f32": F32, "bf16": BF16}[os.environ.get("PH2_DT", "bf16")]
# phase-1 (Wh) matmul dtype; f32r pads the WT rhs to 256 cols for full rate
PH1_DT = {"f32": F32, "bf16": BF16, "f32r": F32R}[os.environ.get("PH1_DT", "bf16")]
WTW = 256 if PH1_DT == F32R else OUT_DIM   # wt width (f32r needs N>=256)
AFT = mybir.ActivationFunctionType
GW = OUT_DIM + 1           # 129 G columns (gWh | g)
GP = 132                   # G pitch (16B aligned)
B1 = 2                     # phase-1 j-tile batch
NB = NJ // B1
HF = RPC // 2              # 512-wide i-halves for phase-2 streams
AG = os.environ.get("AG", "0") == "1"  # all-gather G instead of replicating ph1
NJL = NJ // NCORES         # local j-tiles per core under AG


def emit_body(nc, tc, io, pools):
    at, xt, wt, awr, out = io
    big, atp, ph1, outp = pools

    n_loc = RPC if AG else N       # phase-1 j extent computed locally
    nb_loc = (NJL if AG else NJ) // B1

    # resident loads go via SWDGE so they don't queue behind the HWDGE
    # at-prefetch flood (phase 1 is on their critical path)
    xt_sb = big.tile([P, DH, n_loc], PH1_DT, tag="xt_sb", name="xt_sb")
    nc.gpsimd.dma_start(out=xt_sb, in_=xt.rearrange("(dh p) n -> p dh n", p=P))
    wt_sb = big.tile([P, DH, WTW], PH1_DT, tag="wt_sb", name="wt_sb")
    nc.gpsimd.dma_start(out=wt_sb, in_=wt.rearrange("(dh p) o -> p dh o", p=P))
    aw_sb = big.tile([P, B1, OUT_DIM], F32, tag="aw_sb", name="aw_sb")
    nc.gpsimd.dma_start(out=aw_sb, in_=awr.rearrange("p (b o) -> p b o", b=B1))
    G = big.tile([P, NJ, GP], PH2_DT, tag="G", name="G")
    ones = big.tile([1, P], F32, tag="ones", name="ones")
    nc.vector.memset(ones, 1.0)
    if AG:
        # phase 1 writes the local slice; gathered into G afterwards
        G1 = big.tile([P, NJL, GP], PH2_DT, tag="G1", name="G1")
        dramp = tc.alloc_tile_pool(name="ccd", bufs=1, space="DRAM")
        g_loc = dramp.tile([P, NJL, GP], PH2_DT, name="g_loc")
        g_full = dramp.tile([NCORES * P, NJL, GP], PH2_DT, name="g_full",
                            addr_space="Shared")
    else:
        G1 = G

    with tc.tile_pool(name="ps", bufs=1, space="PSUM") as ps:
        # ---- phase 1: Wh tiles -> e -> g -> G1 = [g*Wh | g], batched by B1 ----
        for b in range(nb_loc):
            wh4 = ps.tile([P, B1, WTW], F32, tag="wh4", name="wh4", bufs=2)
            for k in range(B1):
                t = b * B1 + k
                for dh in range(DH):
                    nc.tensor.matmul(
                        wh4[:, k, :],
                        xt_sb[:, dh, t * P:(t + 1) * P],
                        wt_sb[:, dh, :],
                        start=(dh == 0),
                        stop=(dh == DH - 1),
                    )
            r4 = ph1.tile([P, B1, OUT_DIM], F32, name="r4")
            nc.scalar.activation(r4, wh4[:, :, 0:OUT_DIM], AFT.Relu)
            m4 = ph1.tile([P, B1, OUT_DIM], F32, name="m4")
            nc.vector.tensor_mul(m4, r4, aw_sb)
            e4 = ph1.tile([P, B1], F32, name="e4")
            nc.vector.reduce_sum(e4, m4, axis=mybir.AxisListType.X)
            g4 = ph1.tile([P, B1], F32, name="g4")
            nc.scalar.activation(g4, e4, AFT.Exp)
            for k in range(B1):
                t = b * B1 + k
                nc.vector.tensor_scalar_mul(
                    G1[:, t, 0:OUT_DIM], wh4[:, k, 0:OUT_DIM], g4[:, k:k + 1]
                )
                nc.vector.tensor_copy(
                    out=G1[:, t, OUT_DIM:GW], in_=g4[:, k:k + 1]
                )

        if AG:
            nc.sync.dma_start(out=g_loc, in_=G1)
            nc.gpsimd.collective_compute(
                "AllGather",
                mybir.AluOpType.bypass,
                replica_groups=[list(range(NCORES))],
                ins=[g_loc.opt()],
                outs=[g_full.opt()],
            )
            nc.sync.dma_start(
                out=G.rearrange("p (c t) g -> p c t g", c=NCORES),
                in_=g_full.rearrange("(c p) t g -> p c t g", p=P),
            )

        # ---- phase 2 (transposed): nmT[o, i] += G[jc, o].T @ AT[jc, i] ----
        # numerator rows o=0..127, denominator from the g column (m=1 matmul)
        nm = [ps.tile([P, HF], F32, tag=f"nm{h}", name=f"nm{h}", bufs=1)
              for h in range(2)]
        dn = [ps.tile([P, HF], F32, tag=f"dn{h}", name=f"dn{h}", bufs=1)
              for h in range(2)]
        for c in range(NJ):
            at_sb = atp.tile([P, RPC], PH2_DT, tag="at_sb", name="at_sb")
            nc.sync.dma_start(out=at_sb, in_=at[c * P:(c + 1) * P, :])
            for h in range(2):
                nc.tensor.matmul(
                    nm[h][:, :],
                    G[:, c, 0:OUT_DIM],
                    at_sb[:, h * HF:(h + 1) * HF],
                    start=(c == 0),
                    stop=(c == NJ - 1),
                )
            for h in range(2):
                nc.tensor.matmul(
                    dn[h][0:1, :],
                    G[:, c, OUT_DIM:GW],
                    at_sb[:, h * HF:(h + 1) * HF],
                    start=(c == 0),
                    stop=(c == NJ - 1),
                )
        for h in range(2):
            rc_sb = outp.tile([1, HF], F32, tag="rc", name="rc")
            nc.vector.reciprocal(rc_sb, dn[h][0:1, :])
            rel = outp.tile([P, HF], F32, tag="rel", name="rel")
            nc.scalar.activation(rel, nm[h], AFT.Relu)
            rbc = ps.tile([P, HF], F32, tag="rbc", name="rbc", bufs=1)
            nc.tensor.matmul(rbc, ones[0:1, 0:P], rc_sb, start=True, stop=True)
            o_sb = outp.tile([P, HF], F32, tag="osb", name="osb")
            nc.vector.tensor_mul(o_sb, rel, rbc)
            nc.sync.dma_start(out=out[:, h * HF:(h + 1) * HF], in_=o_sb)


def build_nc(repeat=1):
    nc = bacc.Bacc("TRN2", target_bir_lowering=False,
                   num_devices=NCORES if AG else None)
    at = nc.dram_tensor("at", [N, RPC], PH2_DT, kind="ExternalInput")     # A.T col-block
    xt = nc.dram_tensor("xt", [IN_DIM, RPC if AG else N], PH1_DT,
                        kind="ExternalInput")  # X.T (slice under AG)
    wt = nc.dram_tensor("wt", [IN_DIM, WTW], PH1_DT, kind="ExternalInput")  # W.T (maybe padded)
    awr = nc.dram_tensor("awr", [P, B1 * OUT_DIM], F32, kind="ExternalInput")  # a_w tiled
    out = nc.dram_tensor("out", [OUT_DIM, RPC], F32, kind="ExternalOutput")  # transposed

    with tile.TileContext(nc) as tc:
        with (
            tc.tile_pool(name="big", bufs=1) as big,
            tc.tile_pool(name="atp", bufs=int(os.environ.get("ATBUFS", "20"))) as atp,
            tc.tile_pool(name="ph1", bufs=4) as ph1,
            tc.tile_pool(name="outp", bufs=2) as outp,
        ):
            for _ in range(repeat):
                emit_body(nc, tc, (at, xt, wt, awr, out), (big, atp, ph1, outp))
    nc.compile()
    return nc


_NC_CACHE = None


def _get_nc():
    global _NC_CACHE
    if _NC_CACHE is None:
        _NC_CACHE = build_nc()
    return _NC_CACHE


def make_in_maps(X, A, W, a_w):
    X = np.ascontiguousarray(np.asarray(X, dtype=np.float32))
    A = np.ascontiguousarray(np.asarray(A, dtype=np.float32))
    W = np.ascontiguousarray(np.asarray(W, dtype=np.float32))
    a_w = np.ascontiguousarray(np.asarray(a_w, dtype=np.float32))

    ph1_np = mybir.dt.np(PH1_DT)
    xt_full = np.ascontiguousarray(X.T.astype(ph1_np))   # [256, 8192]
    wt_full = np.zeros((IN_DIM, WTW), dtype=np.float32)
    wt_full[:, :OUT_DIM] = W.T
    wt = np.ascontiguousarray(wt_full.astype(ph1_np))    # [256, WTW]
    awr = np.ascontiguousarray(np.broadcast_to(np.tile(a_w, B1)[None, :], (P, B1 * OUT_DIM)))

    at_np = mybir.dt.np(PH2_DT)
    in_maps = []
    for c in range(NCORES):
        atb = np.ascontiguousarray(A[c * RPC:(c + 1) * RPC, :].T.astype(at_np))
        xt = (np.ascontiguousarray(xt_full[:, c * RPC:(c + 1) * RPC])
              if AG else xt_full)
        in_maps.append({"at": atb, "xt": xt, "wt": wt, "awr": awr})
    return in_maps


def kernel_with_results(X, A, W, a_w, trace=False):
    in_maps = make_in_maps(X, A, W, a_w)
    res = run_bass_kernel_spmd(_get_nc(), in_maps, list(range(NCORES)), trace=trace)
    out = np.concatenate(
        [np.ascontiguousarray(r["out"].T) for r in res.results], axis=0
    )
    return out.astype(np.float32), res


def kernel(X, A, W, a_w):
    out, _ = kernel_with_results(X, A, W, a_w)
    return out


# revision 53
# speedup vs baseline: 1.7057x; 1.7057x over previous
"""GATv2 layer (broadcast-score variant) as a Bass/Tile kernel on 8 NeuronCores.

Math: since scores[i,j] = e[j] (row-broadcast) masked by A, the masked softmax +
aggregation collapse to
    g = exp(e),  e = relu(X @ W.T) @ a_w
    out = relu( (A @ (g*Wh)) / (A @ g) )          with Wh = X @ W.T
Each core computes a 1024-row block of the output:
  phase 1 (replicated): Wh, e, g, G = [g*Wh | g]  ([8192, 129])
  phase 2 (sharded):    acc = A_block @ G  via PE, contraction j on partitions,
                        using the host-transposed A.T block as lhsT.
"""

import numpy as np

import concourse.tile as tile
from concourse import bacc, mybir
from concourse.bass_utils import run_bass_kernel_spmd

N, IN_DIM, OUT_DIM = 8192, 256, 128
NCORES = 8
RPC = N // NCORES          # rows per core (1024)
P = 128                    # partitions
NJ = N // P                # 64 contraction chunks
NI = RPC // P              # 8 output row-tiles per core
DH = IN_DIM // P           # 2 chunks of the d-contraction
import os

F32 = mybir.dt.float32
F32R = mybir.dt.float32r   # TF32-like: 1 cyc/row on PE when moving dim >= 256
BF16 = mybir.dt.bfloat16
# phase-2 matmul dtype: f32r, f32, bf16
PH2_DT = {"f32r": F32R, "f32": F32, "bf16": BF16}[os.environ.get("PH2_DT", "bf16")]
# phase-1 (Wh) matmul dtype; f32r pads the WT rhs to 256 cols for full rate
PH1_DT = {"f32": F32, "bf16": BF16, "f32r": F32R}[os.environ.get("PH1_DT", "bf16")]
WTW = 256 if PH1_DT == F32R else OUT_DIM   # wt width (f32r needs N>=256)
AFT = mybir.ActivationFunctionType
GW = OUT_DIM + 1           # 129 G columns (gWh | g)
GP = 132                   # G pitch (16B aligned)
B1 = 2                     # phase-1 j-tile batch
NB = NJ // B1
HF = RPC // 2              # 512-wide i-halves for phase-2 streams
AG = os.environ.get("AG", "0") == "1"  # all-gather G instead of replicating ph1
NJL = NJ // NCORES         # local j-tiles per core under AG


def emit_body(nc, tc, io, pools):
    at, xt, wt, awr, out = io
    big, atp, ph1, outp = pools

    n_loc = RPC if AG else N       # phase-1 j extent computed locally
    nb_loc = (NJL if AG else NJ) // B1

    xt_sb = big.tile([P, DH, n_loc], PH1_DT, tag="xt_sb", name="xt_sb")
    nc.sync.dma_start(out=xt_sb, in_=xt.rearrange("(dh p) n -> p dh n", p=P))
    wt_sb = big.tile([P, DH, WTW], PH1_DT, tag="wt_sb", name="wt_sb")
    nc.sync.dma_start(out=wt_sb, in_=wt.rearrange("(dh p) o -> p dh o", p=P))
    aw_sb = big.tile([P, B1, OUT_DIM], F32, tag="aw_sb", name="aw_sb")
    nc.sync.dma_start(out=aw_sb, in_=awr.rearrange("p (b o) -> p b o", b=B1))
    G = big.tile([P, NJ, GP], PH2_DT, tag="G", name="G")
    ones = big.tile([1, P], F32, tag="ones", name="ones")
    nc.vector.memset(ones, 1.0)
    if AG:
        # phase 1 writes the local slice; gathered into G afterwards
        G1 = big.tile([P, NJL, GP], PH2_DT, tag="G1", name="G1")
        dramp = tc.alloc_tile_pool(name="ccd", bufs=1, space="DRAM")
        g_loc = dramp.tile([P, NJL, GP], PH2_DT, name="g_loc")
        g_full = dramp.tile([NCORES * P, NJL, GP], PH2_DT, name="g_full",
                            addr_space="Shared")
    else:
        G1 = G

    with tc.tile_pool(name="ps", bufs=1, space="PSUM") as ps:
        # ---- phase 1: Wh tiles -> e -> g -> G1 = [g*Wh | g], batched by B1 ----
        for b in range(nb_loc):
            wh4 = ps.tile([P, B1, WTW], F32, tag="wh4", name="wh4", bufs=2)
            for k in range(B1):
                t = b * B1 + k
                for dh in range(DH):
                    nc.tensor.matmul(
                        wh4[:, k, :],
                        xt_sb[:, dh, t * P:(t + 1) * P],
                        wt_sb[:, dh, :],
                        start=(dh == 0),
                        stop=(dh == DH - 1),
                    )
            r4 = ph1.tile([P, B1, OUT_DIM], F32, name="r4")
            nc.scalar.activation(r4, wh4[:, :, 0:OUT_DIM], AFT.Relu)
            m4 = ph1.tile([P, B1, OUT_DIM], F32, name="m4")
            nc.vector.tensor_mul(m4, r4, aw_sb)
            e4 = ph1.tile([P, B1], F32, name="e4")
            nc.vector.reduce_sum(e4, m4, axis=mybir.AxisListType.X)
            g4 = ph1.tile([P, B1], F32, name="g4")
            nc.scalar.activation(g4, e4, AFT.Exp)
            for k in range(B1):
                t = b * B1 + k
                nc.vector.tensor_scalar_mul(
                    G1[:, t, 0:OUT_DIM], wh4[:, k, 0:OUT_DIM], g4[:, k:k + 1]
                )
                nc.vector.tensor_copy(
                    out=G1[:, t, OUT_DIM:GW], in_=g4[:, k:k + 1]
                )

        if AG:
            nc.sync.dma_start(out=g_loc, in_=G1)
            nc.gpsimd.collective_compute(
                "AllGather",
                mybir.AluOpType.bypass,
                replica_groups=[list(range(NCORES))],
                ins=[g_loc.opt()],
                outs=[g_full.opt()],
            )
            nc.sync.dma_start(
                out=G.rearrange("p (c t) g -> p c t g", c=NCORES),
                in_=g_full.rearrange("(c p) t g -> p c t g", p=P),
            )

        # ---- phase 2 (transposed): nmT[o, i] += G[jc, o].T @ AT[jc, i] ----
        # numerator rows o=0..127, denominator from the g column (m=1 matmul)
        nm = [ps.tile([P, HF], F32, tag=f"nm{h}", name=f"nm{h}", bufs=1)
              for h in range(2)]
        dn = [ps.tile([P, HF], F32, tag=f"dn{h}", name=f"dn{h}", bufs=1)
              for h in range(2)]
        for c in range(NJ):
            at_sb = atp.tile([P, RPC], PH2_DT, tag="at_sb", name="at_sb")
            nc.sync.dma_start(out=at_sb, in_=at[c * P:(c + 1) * P, :])
            for h in range(2):
                nc.tensor.matmul(
                    nm[h][:, :],
                    G[:, c, 0:OUT_DIM],
                    at_sb[:, h * HF:(h + 1) * HF],
                    start=(c == 0),
                    stop=(c == NJ - 1),
                )
            for h in range(2):
                nc.tensor.matmul(
                    dn[h][0:1, :],
                    G[:, c, OUT_DIM:GW],
                    at_sb[:, h * HF:(h + 1) * HF],
                    start=(c == 0),
                    stop=(c == NJ - 1),
                )
        for h in range(2):
            rc_sb = outp.tile([1, HF], F32, tag="rc", name="rc")
            nc.vector.reciprocal(rc_sb, dn[h][0:1, :])
            rel = outp.tile([P, HF], F32, tag="rel", name="rel")
            nc.scalar.activation(rel, nm[h], AFT.Relu)
            rbc = ps.tile([P, HF], F32, tag="rbc", name="rbc", bufs=1)
            nc.tensor.matmul(rbc, ones[0:1, 0:P], rc_sb, start=True, stop=True)
            o_sb = outp.tile([P, HF], F32, tag="osb", name="osb")
            nc.vector.tensor_mul(o_sb, rel, rbc)
            nc.sync.dma_start(out=out[:, h * HF:(h + 1) * HF], in_=o_sb)


def build_nc(repeat=1):
    nc = bacc.Bacc("TRN2", target_bir_lowering=False,
                   num_devices=NCORES if AG else None)
    at = nc.dram_tensor("at", [N, RPC], PH2_DT, kind="ExternalInput")     # A.T col-block
    xt = nc.dram_tensor("xt", [IN_DIM, RPC if AG else N], PH1_DT,
                        kind="ExternalInput")  # X.T (slice under AG)
    wt = nc.dram_tensor("wt", [IN_DIM, WTW], PH1_DT, kind="ExternalInput")  # W.T (maybe padded)
    awr = nc.dram_tensor("awr", [P, B1 * OUT_DIM], F32, kind="ExternalInput")  # a_w tiled
    out = nc.dram_tensor("out", [OUT_DIM, RPC], F32, kind="ExternalOutput")  # transposed

    with tile.TileContext(nc) as tc:
        with (
            tc.tile_pool(name="big", bufs=1) as big,
            tc.tile_pool(name="atp", bufs=int(os.environ.get("ATBUFS", "20"))) as atp,
            tc.tile_pool(name="ph1", bufs=4) as ph1,
            tc.tile_pool(name="outp", bufs=2) as outp,
        ):
            for _ in range(repeat):
                emit_body(nc, tc, (at, xt, wt, awr, out), (big, atp, ph1, outp))
    nc.compile()
    return nc


_NC_CACHE = None


def _get_nc():
    global _NC_CACHE
    if _NC_CACHE is None:
        _NC_CACHE = build_nc()
    return _NC_CACHE


def make_in_maps(X, A, W, a_w):
    X = np.ascontiguousarray(np.asarray(X, dtype=np.float32))
    A = np.ascontiguousarray(np.asarray(A, dtype=np.float32))
    W = np.ascontiguousarray(np.asarray(W, dtype=np.float32))
    a_w = np.ascontiguousarray(np.asarray(a_w, dtype=np.float32))

    ph1_np = mybir.dt.np(PH1_DT)
    xt_full = np.ascontiguousarray(X.T.astype(ph1_np))   # [256, 8192]
    wt_full = np.zeros((IN_DIM, WTW), dtype=np.float32)
    wt_full[:, :OUT_DIM] = W.T
    wt = np.ascontiguousarray(wt_full.astype(ph1_np))    # [256, WTW]
    awr = np.ascontiguousarray(np.broadcast_to(np.tile(a_w, B1)[None, :], (P, B1 * OUT_DIM)))

    at_np = mybir.dt.np(PH2_DT)
    in_maps = []
    for c in range(NCORES):
        atb = np.ascontiguousarray(A[c * RPC:(c + 1) * RPC, :].T.astype(at_np))
        xt = (np.ascontiguousarray(xt_full[:, c * RPC:(c + 1) * RPC])
              if AG else xt_full)
        in_maps.append({"at": atb, "xt": xt, "wt": wt, "awr": awr})
    return in_maps


def kernel_with_results(X, A, W, a_w, trace=False):
    in_maps = make_in_maps(X, A, W, a_w)
    res = run_bass_kernel_spmd(_get_nc(), in_maps, list(range(NCORES)), trace=trace)
    out = np.concatenate(
        [np.ascontiguousarray(r["out"].T) for r in res.results], axis=0
    )
    return out.astype(np.float32), res


def kernel(X, A, W, a_w):
    out, _ = kernel_with_results(X, A, W, a_w)
    return out
